# revision 13
# baseline (speedup 1.0000x reference)
"""Trainium2 Bass kernel for nn_MixtureOfExpertsES (moe_routing).

Expert-parallel over 8 NeuronCores: core c owns expert c (W1[c]/W2[c]
resident in SBUF as f32r). Each core receives the full token set as
X^T [DM, S], computes gate weights for its own expert on-device
(fp32 logits -> top-2-of-8 renormalized softmax weights), runs the
dense FFN for its expert over all tokens in f32r (TF32-class matmuls),
scales by the gate weight (zero for tokens that didn't pick this
expert), and the partial outputs Y^T are summed with an on-device
ReduceScatter. Core c returns rows [c*96:(c+1)*96] of the summed
Y^T [768, 4096]; the host concatenates and transposes back.
"""
import sys

if '/opt/trn_rl_repo' not in sys.path:
    sys.path.insert(0, '/opt/trn_rl_repo')

import numpy as np

B, T, DM, DF, E = 4, 1024, 768, 3072, 8
S = B * T                      # 4096 tokens
N_CORES = 8
CHUNK = 256                    # tokens per FFN chunk (f32r wants moving dim >= 256)
NBLK = CHUNK // 128            # token blocks per chunk
NCH = S // CHUNK               # chunks
KD = DM // 128                 # 6 k-subtiles over DM
KF = DF // 128                 # 24 k-subtiles over DF
OUT_ROWS = DM // N_CORES       # 96 rows of Y^T per core after reduce-scatter

_built = None
LAST_RESULTS = None            # BassKernelResults of the most recent run (for test.py)


def build_moe(num_devices=N_CORES, debug=False, with_collective=True):
    import concourse.mybir as mybir
    import concourse.tile as tile
    from concourse import bacc
    from concourse.masks import make_identity

    f32 = mybir.dt.float32
    f32r = mybir.dt.float32r
    ACT = mybir.ActivationFunctionType
    ALU = mybir.AluOpType

    nc = bacc.Bacc("TRN2", target_bir_lowering=False, debug=False,
                   num_devices=num_devices)

    xt_d = nc.dram_tensor("xt", [DM, S], f32, kind="ExternalInput").ap()
    wg_d = nc.dram_tensor("wg", [DM, E], f32, kind="ExternalInput").ap()
    w1_d = nc.dram_tensor("w1", [DM, DF], f32r, kind="ExternalInput").ap()
    w2_d = nc.dram_tensor("w2", [DF, DM], f32r, kind="ExternalInput").ap()
    b1_d = nc.dram_tensor("b1c", [128, KF], f32, kind="ExternalInput").ap()
    b2_d = nc.dram_tensor("b2c", [128, KD], f32, kind="ExternalInput").ap()
    sel_d = nc.dram_tensor("sel", [128, E], f32, kind="ExternalInput").ap()
    if with_collective:
        out_d = nc.dram_tensor("out", [OUT_ROWS, S], f32, kind="ExternalOutput").ap()
    else:
        out_d = nc.dram_tensor("out", [DM, S], f32, kind="ExternalOutput").ap()
    if debug:
        dbg_l = nc.dram_tensor("dbg_l", [S, E], f32, kind="ExternalOutput").ap()
        dbg_g = nc.dram_tensor("dbg_g", [S], f32, kind="ExternalOutput").ap()
        dbg_gb = nc.dram_tensor("dbg_gb", [128, CHUNK], f32, kind="ExternalOutput").ap()
        dbg_mx = nc.dram_tensor("dbg_mx", [S, 8], f32, kind="ExternalOutput").ap()

    with tile.TileContext(nc) as tc:
        with (
            tc.tile_pool(name="wpool", bufs=1) as wpool,
            tc.tile_pool(name="xpool", bufs=2) as xpool,
            tc.tile_pool(name="hpool", bufs=1) as hpool,
            tc.tile_pool(name="gpool", bufs=2) as gpool,
            tc.tile_pool(name="ypool", bufs=3) as ypool,
            tc.tile_pool(name="spool", bufs=1) as spool,
            tc.tile_pool(name="psA", bufs=2, space="PSUM") as psA,
            tc.tile_pool(name="psB", bufs=2, space="PSUM") as psB,
            tc.tile_pool(name="psG", bufs=1, space="PSUM") as psG,
            tc.tile_pool(name="psT", bufs=1, space="PSUM") as psT,
            tc.tile_pool(name="dram", bufs=1, space="DRAM") as dram,
        ):
            # resident weights, k-subtiled with the contraction dim on partitions
            w1_sb = wpool.tile([128, KD, DF], f32r)
            nc.sync.dma_start(w1_sb[:], w1_d.rearrange("(o p) f -> p o f", p=128))
            w2_sb = wpool.tile([128, KF, DM], f32r)
            nc.sync.dma_start(w2_sb[:], w2_d.rearrange("(o p) m -> p o m", p=128))
            wg_sb = spool.tile([128, KD, E], f32)
            nc.sync.dma_start(wg_sb[:], wg_d.rearrange("(o p) e -> p o e", p=128))
            sel_sb = spool.tile([128, E], f32)
            nc.sync.dma_start(sel_sb[:], sel_d)
            b1_sb = spool.tile([128, KF], f32)
            nc.sync.dma_start(b1_sb[:], b1_d)
            b2_sb = spool.tile([128, KD], f32)
            nc.sync.dma_start(b2_sb[:], b2_d)
            ident = spool.tile([128, 128], f32)
            make_identity(nc, ident[:])

            RS_SPLIT = 4
            BAND = S // RS_SPLIT           # tokens per collective band
            yt_bands = [dram.tile([DM, BAND], f32, name=f"ytb{i}")
                        for i in range(RS_SPLIT)]
            rs_bands = [dram.tile([OUT_ROWS, BAND], f32, name=f"rsb{i}")
                        for i in range(RS_SPLIT)]

            for c in range(NCH):
                t0 = c * CHUNK
                # full-precision copy for the gate logits (f32r DMA rounds!)
                xt_sb = xpool.tile([128, KD, CHUNK], f32, tag="xt")
                nc.sync.dma_start(
                    xt_sb[:],
                    xt_d[:, t0:t0 + CHUNK].rearrange("(o p) t -> p o t", p=128))
                # rounded copy for the FFN matmuls
                xtr_sb = xpool.tile([128, KD, CHUNK], f32r, tag="xtr")
                nc.vector.tensor_copy(xtr_sb[:], xt_sb[:])

                # ---- gating: fp32 logits -> per-token weight for this expert
                l_sb = gpool.tile([128, NBLK, E], f32, tag="l")
                for b in range(NBLK):
                    ps_g = psG.tile([128, E], f32)
                    for k in range(KD):
                        nc.tensor.matmul(
                            ps_g[:],
                            xt_sb[:, k, b * 128:(b + 1) * 128],
                            wg_sb[:, k, :],
                            start=(k == 0), stop=(k == KD - 1))
                    nc.scalar.activation(l_sb[:, b, :], ps_g[:], ACT.Copy)
                mx = gpool.tile([128, NBLK, 8], f32, tag="mx")
                for b in range(NBLK):
                    nc.vector.max(mx[:, b, :], l_sb[:, b, :])
                m1 = mx[:, :, 0]
                m2 = mx[:, :, 1]
                tmp = gpool.tile([128, NBLK, E], f32, tag="tmp")
                nc.vector.tensor_tensor(
                    tmp[:], l_sb[:],
                    sel_sb[:, None, :].to_broadcast((128, NBLK, E)), ALU.mult)
                le = gpool.tile([128, NBLK], f32, tag="le")
                nc.vector.tensor_reduce(le[:], tmp[:], mybir.AxisListType.X, ALU.add)
                keep = gpool.tile([128, NBLK], f32, tag="keep")
                nc.vector.tensor_tensor(keep[:], le[:], m2, ALU.is_ge)
                d21 = gpool.tile([128, NBLK], f32, tag="d21")
                nc.vector.tensor_tensor(d21[:], m2, m1, ALU.subtract)
                nc.scalar.activation(d21[:], d21[:], ACT.Exp)
                nc.vector.tensor_scalar_add(d21[:], d21[:], 1.0)
                inv = gpool.tile([128, NBLK], f32, tag="inv")
                nc.vector.reciprocal(inv[:], d21[:])
                g_sb = gpool.tile([128, NBLK], f32, tag="g")
                nc.vector.tensor_tensor(g_sb[:], le[:], m1, ALU.subtract)
                nc.scalar.activation(g_sb[:], g_sb[:], ACT.Exp)
                nc.vector.tensor_tensor(g_sb[:], g_sb[:], keep, ALU.mult)
                nc.vector.tensor_tensor(g_sb[:], g_sb[:], inv, ALU.mult)
                # broadcast g across partitions: transpose the free-broadcast
                # column [128, 128] so every partition row holds g(token)
                gb = gpool.tile([128, CHUNK], f32, tag="gb")
                for b in range(NBLK):
                    ps_t = psT.tile([128, 128], f32)
                    nc.tensor.transpose(
                        ps_t[:], g_sb[:, b:b + 1].to_broadcast((128, 128)), ident[:])
                    nc.scalar.activation(
                        gb[:, b * 128:(b + 1) * 128], ps_t[:], ACT.Copy)

                if debug:
                    nc.sync.dma_start(
                        dbg_l[t0:t0 + CHUNK, :].rearrange("(o p) e -> p o e", p=128),
                        l_sb[:])
                    nc.sync.dma_start(
                        dbg_mx[t0:t0 + CHUNK, :].rearrange("(o p) e -> p o e", p=128),
                        mx[:])
                    nc.sync.dma_start(
                        dbg_g[t0:t0 + CHUNK].rearrange("(o p) -> p o", p=128),
                        g_sb[:])
                    if c == 0:
                        nc.sync.dma_start(dbg_gb, gb[:])
                del g_sb

                # ---- FFN: H^T = relu(W1^T X^T + b1), Y^T = g * (W2^T H^T + b2)
                ht_sb = hpool.tile([128, KF, CHUNK], f32r, tag="ht")
                for m in range(KF):
                    ps = psA.tile([128, CHUNK], f32)
                    for k in range(KD):
                        nc.tensor.matmul(
                            ps[:],
                            w1_sb[:, k, m * 128:(m + 1) * 128],
                            xtr_sb[:, k, :],
                            start=(k == 0), stop=(k == KD - 1))
                    # bias + relu + round-to-f32r in one DVE op (ACT cannot
                    # write f32r on HW)
                    nc.vector.tensor_scalar(
                        ht_sb[:, m, :], ps[:], b1_sb[:, m:m + 1], 0.0,
                        ALU.add, ALU.max)
                for m in range(KD):
                    ps = psB.tile([128, CHUNK], f32)
                    for k in range(KF):
                        nc.tensor.matmul(
                            ps[:],
                            w2_sb[:, k, m * 128:(m + 1) * 128],
                            ht_sb[:, k, :],
                            start=(k == 0), stop=(k == KF - 1))
                    yt = ypool.tile([128, CHUNK], f32, tag="yt")
                    nc.scalar.activation(yt[:], ps[:], ACT.Identity,
                                         bias=b2_sb[:, m:m + 1], scale=1.0)
                    nc.vector.tensor_tensor(yt[:], yt[:], gb[:], ALU.mult)
                    nc.sync.dma_start(
                        yt_dram[m * 128:(m + 1) * 128, t0:t0 + CHUNK], yt[:])

            if with_collective:
                nc.gpsimd.collective_compute(
                    "ReduceScatter",
                    mybir.AluOpType.add,
                    replica_groups=[list(range(num_devices))],
                    ins=[yt_dram.opt()],
                    outs=[rs_out.opt()],
                )
                nc.sync.dma_start(out_d, rs_out[:])
            else:
                nc.sync.dma_start(out_d, yt_dram[:])

    nc.compile()
    return nc


def make_in_map(x, Wg, W1, b1, W2, b2, e):
    xt = np.ascontiguousarray(x.reshape(S, DM).T)          # [DM, S]
    sel = np.zeros((128, E), np.float32)
    sel[:, e] = 1.0
    return dict(
        xt=xt,
        wg=np.ascontiguousarray(Wg),
        w1=np.ascontiguousarray(W1[e]),
        w2=np.ascontiguousarray(W2[e]),
        b1c=np.ascontiguousarray(b1[e].reshape(KF, 128).T),
        b2c=np.ascontiguousarray(b2[e].reshape(KD, 128).T),
        sel=sel,
    )


def kernel(x, Wg, W1, b1, W2, b2):
    global _built, LAST_RESULTS
    from concourse import bass_utils

    x = np.asarray(x, np.float32)
    Wg = np.asarray(Wg, np.float32)
    W1 = np.asarray(W1, np.float32)
    b1 = np.asarray(b1, np.float32)
    W2 = np.asarray(W2, np.float32)
    b2 = np.asarray(b2, np.float32)

    if _built is None:
        _built = build_moe()
    nc = _built

    in_maps = [make_in_map(x, Wg, W1, b1, W2, b2, e) for e in range(N_CORES)]
    res = bass_utils.run_bass_kernel_spmd(nc, in_maps, core_ids=list(range(N_CORES)))
    LAST_RESULTS = res
    yt = np.concatenate([res.results[c]["out"] for c in range(N_CORES)], axis=0)
    return np.ascontiguousarray(yt.T).reshape(B, T, DM).astype(np.float32)


# revision 15
# speedup vs baseline: 1.0816x; 1.0816x over previous
"""Trainium2 Bass kernel for nn_MixtureOfExpertsES (moe_routing).

Expert-parallel over 8 NeuronCores: core c owns expert c (W1[c]/W2[c]
resident in SBUF as f32r). Each core receives the full token set as
X^T [DM, S], computes gate weights for its own expert on-device
(fp32 logits -> top-2-of-8 renormalized softmax weights), runs the
dense FFN for its expert over all tokens in f32r (TF32-class matmuls),
scales by the gate weight (zero for tokens that didn't pick this
expert), and the partial outputs Y^T are summed with an on-device
ReduceScatter. Core c returns rows [c*96:(c+1)*96] of the summed
Y^T [768, 4096]; the host concatenates and transposes back.
"""
import sys

if '/opt/trn_rl_repo' not in sys.path:
    sys.path.insert(0, '/opt/trn_rl_repo')

import numpy as np

B, T, DM, DF, E = 4, 1024, 768, 3072, 8
S = B * T                      # 4096 tokens
N_CORES = 8
CHUNK = 256                    # tokens per FFN chunk (f32r wants moving dim >= 256)
NBLK = CHUNK // 128            # token blocks per chunk
NCH = S // CHUNK               # chunks
KD = DM // 128                 # 6 k-subtiles over DM
KF = DF // 128                 # 24 k-subtiles over DF
OUT_ROWS = DM // N_CORES       # 96 rows of Y^T per core after reduce-scatter

_built = None
LAST_RESULTS = None            # BassKernelResults of the most recent run (for test.py)


def build_moe(num_devices=N_CORES, debug=False, with_collective=True):
    import concourse.mybir as mybir
    import concourse.tile as tile
    from concourse import bacc
    from concourse.masks import make_identity

    f32 = mybir.dt.float32
    f32r = mybir.dt.float32r
    ACT = mybir.ActivationFunctionType
    ALU = mybir.AluOpType

    nc = bacc.Bacc("TRN2", target_bir_lowering=False, debug=False,
                   num_devices=num_devices)

    xt_d = nc.dram_tensor("xt", [DM, S], f32, kind="ExternalInput").ap()
    wg_d = nc.dram_tensor("wg", [DM, E], f32, kind="ExternalInput").ap()
    w1_d = nc.dram_tensor("w1", [DM, DF], f32r, kind="ExternalInput").ap()
    w2_d = nc.dram_tensor("w2", [DF, DM], f32r, kind="ExternalInput").ap()
    b1_d = nc.dram_tensor("b1c", [128, KF], f32, kind="ExternalInput").ap()
    b2_d = nc.dram_tensor("b2c", [128, KD], f32, kind="ExternalInput").ap()
    sel_d = nc.dram_tensor("sel", [128, E], f32, kind="ExternalInput").ap()
    if with_collective:
        out_d = nc.dram_tensor("out", [OUT_ROWS, S], f32, kind="ExternalOutput").ap()
    else:
        out_d = nc.dram_tensor("out", [DM, S], f32, kind="ExternalOutput").ap()
    if debug:
        dbg_l = nc.dram_tensor("dbg_l", [S, E], f32, kind="ExternalOutput").ap()
        dbg_g = nc.dram_tensor("dbg_g", [S], f32, kind="ExternalOutput").ap()
        dbg_gb = nc.dram_tensor("dbg_gb", [128, CHUNK], f32, kind="ExternalOutput").ap()
        dbg_mx = nc.dram_tensor("dbg_mx", [S, 8], f32, kind="ExternalOutput").ap()

    with tile.TileContext(nc) as tc:
        with (
            tc.tile_pool(name="wpool", bufs=1) as wpool,
            tc.tile_pool(name="xpool", bufs=2) as xpool,
            tc.tile_pool(name="hpool", bufs=1) as hpool,
            tc.tile_pool(name="gpool", bufs=2) as gpool,
            tc.tile_pool(name="ypool", bufs=3) as ypool,
            tc.tile_pool(name="spool", bufs=1) as spool,
            tc.tile_pool(name="psA", bufs=2, space="PSUM") as psA,
            tc.tile_pool(name="psB", bufs=2, space="PSUM") as psB,
            tc.tile_pool(name="psG", bufs=1, space="PSUM") as psG,
            tc.tile_pool(name="psT", bufs=1, space="PSUM") as psT,
            tc.tile_pool(name="dram", bufs=1, space="DRAM") as dram,
        ):
            # resident weights, k-subtiled with the contraction dim on partitions
            w1_sb = wpool.tile([128, KD, DF], f32r)
            nc.sync.dma_start(w1_sb[:], w1_d.rearrange("(o p) f -> p o f", p=128))
            w2_sb = wpool.tile([128, KF, DM], f32r)
            nc.sync.dma_start(w2_sb[:], w2_d.rearrange("(o p) m -> p o m", p=128))
            wg_sb = spool.tile([128, KD, E], f32)
            nc.sync.dma_start(wg_sb[:], wg_d.rearrange("(o p) e -> p o e", p=128))
            sel_sb = spool.tile([128, E], f32)
            nc.sync.dma_start(sel_sb[:], sel_d)
            b1_sb = spool.tile([128, KF], f32)
            nc.sync.dma_start(b1_sb[:], b1_d)
            b2_sb = spool.tile([128, KD], f32)
            nc.sync.dma_start(b2_sb[:], b2_d)
            ident = spool.tile([128, 128], f32)
            make_identity(nc, ident[:])

            RS_SPLIT = 4
            BAND = S // RS_SPLIT           # tokens per collective band
            yt_bands = [dram.tile([DM, BAND], f32, name=f"ytb{i}")
                        for i in range(RS_SPLIT)]
            rs_bands = [dram.tile([OUT_ROWS, BAND], f32, name=f"rsb{i}")
                        for i in range(RS_SPLIT)]

            for c in range(NCH):
                t0 = c * CHUNK
                # full-precision copy for the gate logits (f32r DMA rounds!)
                xt_sb = xpool.tile([128, KD, CHUNK], f32, tag="xt")
                nc.sync.dma_start(
                    xt_sb[:],
                    xt_d[:, t0:t0 + CHUNK].rearrange("(o p) t -> p o t", p=128))
                # rounded copy for the FFN matmuls
                xtr_sb = xpool.tile([128, KD, CHUNK], f32r, tag="xtr")
                nc.vector.tensor_copy(xtr_sb[:], xt_sb[:])

                # ---- gating: fp32 logits -> per-token weight for this expert
                l_sb = gpool.tile([128, NBLK, E], f32, tag="l")
                for b in range(NBLK):
                    ps_g = psG.tile([128, E], f32)
                    for k in range(KD):
                        nc.tensor.matmul(
                            ps_g[:],
                            xt_sb[:, k, b * 128:(b + 1) * 128],
                            wg_sb[:, k, :],
                            start=(k == 0), stop=(k == KD - 1))
                    nc.scalar.activation(l_sb[:, b, :], ps_g[:], ACT.Copy)
                mx = gpool.tile([128, NBLK, 8], f32, tag="mx")
                for b in range(NBLK):
                    nc.vector.max(mx[:, b, :], l_sb[:, b, :])
                m1 = mx[:, :, 0]
                m2 = mx[:, :, 1]
                tmp = gpool.tile([128, NBLK, E], f32, tag="tmp")
                nc.vector.tensor_tensor(
                    tmp[:], l_sb[:],
                    sel_sb[:, None, :].to_broadcast((128, NBLK, E)), ALU.mult)
                le = gpool.tile([128, NBLK], f32, tag="le")
                nc.vector.tensor_reduce(le[:], tmp[:], mybir.AxisListType.X, ALU.add)
                keep = gpool.tile([128, NBLK], f32, tag="keep")
                nc.vector.tensor_tensor(keep[:], le[:], m2, ALU.is_ge)
                d21 = gpool.tile([128, NBLK], f32, tag="d21")
                nc.vector.tensor_tensor(d21[:], m2, m1, ALU.subtract)
                nc.scalar.activation(d21[:], d21[:], ACT.Exp)
                nc.vector.tensor_scalar_add(d21[:], d21[:], 1.0)
                inv = gpool.tile([128, NBLK], f32, tag="inv")
                nc.vector.reciprocal(inv[:], d21[:])
                g_sb = gpool.tile([128, NBLK], f32, tag="g")
                nc.vector.tensor_tensor(g_sb[:], le[:], m1, ALU.subtract)
                nc.scalar.activation(g_sb[:], g_sb[:], ACT.Exp)
                nc.vector.tensor_tensor(g_sb[:], g_sb[:], keep, ALU.mult)
                nc.vector.tensor_tensor(g_sb[:], g_sb[:], inv, ALU.mult)
                # broadcast g across partitions: transpose the free-broadcast
                # column [128, 128] so every partition row holds g(token)
                gb = gpool.tile([128, CHUNK], f32, tag="gb")
                for b in range(NBLK):
                    ps_t = psT.tile([128, 128], f32)
                    nc.tensor.transpose(
                        ps_t[:], g_sb[:, b:b + 1].to_broadcast((128, 128)), ident[:])
                    nc.scalar.activation(
                        gb[:, b * 128:(b + 1) * 128], ps_t[:], ACT.Copy)

                if debug:
                    nc.sync.dma_start(
                        dbg_l[t0:t0 + CHUNK, :].rearrange("(o p) e -> p o e", p=128),
                        l_sb[:])
                    nc.sync.dma_start(
                        dbg_mx[t0:t0 + CHUNK, :].rearrange("(o p) e -> p o e", p=128),
                        mx[:])
                    nc.sync.dma_start(
                        dbg_g[t0:t0 + CHUNK].rearrange("(o p) -> p o", p=128),
                        g_sb[:])
                    if c == 0:
                        nc.sync.dma_start(dbg_gb, gb[:])
                del g_sb

                # ---- FFN: H^T = relu(W1^T X^T + b1), Y^T = g * (W2^T H^T + b2)
                ht_sb = hpool.tile([128, KF, CHUNK], f32r, tag="ht")
                for m in range(KF):
                    ps = psA.tile([128, CHUNK], f32)
                    for k in range(KD):
                        nc.tensor.matmul(
                            ps[:],
                            w1_sb[:, k, m * 128:(m + 1) * 128],
                            xtr_sb[:, k, :],
                            start=(k == 0), stop=(k == KD - 1))
                    # bias + relu + round-to-f32r in one DVE op (ACT cannot
                    # write f32r on HW)
                    nc.vector.tensor_scalar(
                        ht_sb[:, m, :], ps[:], b1_sb[:, m:m + 1], 0.0,
                        ALU.add, ALU.max)
                for m in range(KD):
                    ps = psB.tile([128, CHUNK], f32)
                    for k in range(KF):
                        nc.tensor.matmul(
                            ps[:],
                            w2_sb[:, k, m * 128:(m + 1) * 128],
                            ht_sb[:, k, :],
                            start=(k == 0), stop=(k == KF - 1))
                    yt = ypool.tile([128, CHUNK], f32, tag="yt")
                    nc.scalar.activation(yt[:], ps[:], ACT.Identity,
                                         bias=b2_sb[:, m:m + 1], scale=1.0)
                    nc.vector.tensor_tensor(yt[:], yt[:], gb[:], ALU.mult)
                    band = c // (NCH // RS_SPLIT)
                    col = t0 - band * BAND
                    nc.sync.dma_start(
                        yt_bands[band][m * 128:(m + 1) * 128, col:col + CHUNK],
                        yt[:])

                # fire the band's reduce-scatter as soon as its last chunk is out
                if with_collective and (c + 1) % (NCH // RS_SPLIT) == 0:
                    band = c // (NCH // RS_SPLIT)
                    nc.gpsimd.collective_compute(
                        "ReduceScatter",
                        mybir.AluOpType.add,
                        replica_groups=[list(range(num_devices))],
                        ins=[yt_bands[band].opt()],
                        outs=[rs_bands[band].opt()],
                    )
                    nc.sync.dma_start(
                        out_d[:, band * BAND:(band + 1) * BAND],
                        rs_bands[band][:])

            if not with_collective:
                for band in range(RS_SPLIT):
                    nc.sync.dma_start(
                        out_d[:, band * BAND:(band + 1) * BAND],
                        yt_bands[band][:])

    nc.compile()
    return nc


def make_in_map(x, Wg, W1, b1, W2, b2, e):
    xt = np.ascontiguousarray(x.reshape(S, DM).T)          # [DM, S]
    sel = np.zeros((128, E), np.float32)
    sel[:, e] = 1.0
    return dict(
        xt=xt,
        wg=np.ascontiguousarray(Wg),
        w1=np.ascontiguousarray(W1[e]),
        w2=np.ascontiguousarray(W2[e]),
        b1c=np.ascontiguousarray(b1[e].reshape(KF, 128).T),
        b2c=np.ascontiguousarray(b2[e].reshape(KD, 128).T),
        sel=sel,
    )


def kernel(x, Wg, W1, b1, W2, b2):
    global _built, LAST_RESULTS
    from concourse import bass_utils

    x = np.asarray(x, np.float32)
    Wg = np.asarray(Wg, np.float32)
    W1 = np.asarray(W1, np.float32)
    b1 = np.asarray(b1, np.float32)
    W2 = np.asarray(W2, np.float32)
    b2 = np.asarray(b2, np.float32)

    if _built is None:
        _built = build_moe()
    nc = _built

    in_maps = [make_in_map(x, Wg, W1, b1, W2, b2, e) for e in range(N_CORES)]
    res = None
    for attempt in range(3):
        try:
            res = bass_utils.run_bass_kernel_spmd(
                nc, in_maps, core_ids=list(range(N_CORES)))
            break
        except Exception:
            # the runtime occasionally reports a transient
            # NRT_EXEC_UNIT_UNRECOVERABLE; a fresh execute recovers it
            if attempt == 2:
                raise
    LAST_RESULTS = res
    yt = np.concatenate([res.results[c]["out"] for c in range(N_CORES)], axis=0)
    return np.ascontiguousarray(yt.T).reshape(B, T, DM).astype(np.float32)
